# revision 3
# baseline (speedup 1.0000x reference)
"""DTW layer (short kernel) Trainium2 Bass kernel — custom-DVE-op version.

Problem: x (B=8, C=8, L=4096) f32, kernels (F=32, K=10) f32.
For each (b, c, f, w): DTW cost between kernels[f] (len 10) and window
x[b, c, 5w : 5w+20], w in [0, 815).  Output (B, C*F, 815) f32.

Sharding: data-parallel over batch — core b computes batch b entirely
(C*F = 256 (c,f) combos = 2 partition chunks of 128).

Algorithm: within one window the DTW row update
    acc[j] = d[j] + min(m[j], acc[j-1]),  acc[-1] = +inf
unrolls to acc[j] = min_{i<=j}(m[i] - P[i-1]) + P[j], P = prefix sum of
d.  All windows of a chunk are streamed back-to-back (20 cells per
window, NO separators).  Window isolation comes from a staircase: the
carried row values Y = acc - w*GAMMA drop by GAMMA per window, so one
global min-scan never lets an earlier window undercut a later one (the
global prefix sum P is monotone and only strengthens the isolation).

One custom DVE instruction (DTW_ROW_ANT) computes a whole row at
1 elem/cycle — vs 2 cycles/elem for the stock tensor_tensor_scan —
with the local costs fused in (ScalarE squares eliminated):
    d   = (x + C0)^2           C0 = -kernel[i]   (per-partition scalar)
    P   = scan(ADD, d)
    t   = (m~ - P) + d         = m~[k] - P[k-1]
    U   = scan(MIN, t, C1)     C1 = 1e30
    out = U + P                = Y_i[k] = acc_i[k] - w*GAMMA
The MIN-scan's expr references the ADD-scan; dve_spec's scan() forbids
that syntactically but the 8-stage datapath handles it (each scan stage
has its own CURR_ALU_OUT feedback flop), so the Scan node is built
directly, bypassing __post_init__.  Verified bit-exact on HW.

Between rows, m~_i[k] = min(Y[k], Y[k-1]) is one stock tensor_tensor
min over two shifted views of Y (a guard column = BIG covers k=0; the
staircase makes the window-crossing positions come out right).  Row 0
uses a constant m~0 (= -w*GAMMA at each window start, BIG elsewhere).
A second tiny custom op ADD_RAMP (out = in + Idx*C0) builds m~0 and
undoes the staircase when extracting column j=19 of each window.

Raw bass (no Tile framework): this toolchain's walrus codegen allows at
most 2 embedded sync-waits per instruction, so engines are programmed
directly with standalone wait_ge and per-engine semaphores.
`lower_extended_insts` must run after tracing (raw Bass skips Bacc's
codegen_inst_isa_subclasses pass; without it custom-DVE instructions
have empty .instr bytes and walrus fails with "ISA wrong length").
"""

from contextlib import ExitStack

import numpy as np

import concourse.bass as bass
import concourse.mybir as mybir
from concourse.bass_utils import run_bass_kernel_spmd
from concourse.library_overlay import lower_extended_insts

# Problem constants (hardcoded per harness contract)
B, C, L = 8, 8, 4096
F, K = 32, 10
PROC, STEP = 20, 5
NW = 815          # windows actually computed == chan_outlen
NWC = 136         # windows per chunk; 6 chunks = 816 >= 815
NCHUNK = 6
TFREE = NWC * PROC  # 2720 stream length per chunk
BIG = 1e30
GAMMA = 4096.0    # staircase step; power of 2 so w*GAMMA is exact
SLOTS = 2
UNITS = [(cc, wc) for cc in range(2) for wc in range(NCHUNK)]

F32 = mybir.dt.float32


# --- custom DVE ops -------------------------------------------------------- #

def _register_ops():
    import concourse.dve_ops as dom
    from concourse.dve_ops import DveOp
    from concourse.dve_spec import (
        C0, C1, AluOp, Idx, Scan, Spec, Src0, Src1, _has_src1, lower, scan,
        sq,
    )
    from concourse.dve_uop import DveOpSpec

    def raw_scan(op, expr, init=None):
        s = object.__new__(Scan)
        object.__setattr__(s, "op", op)
        object.__setattr__(s, "expr", expr)
        object.__setattr__(s, "init", init)
        object.__setattr__(s, "_subdim_step", None)
        return s

    def dtw_row_ref(in0, in1, s0, s1, imm2):
        p = in0.shape[0]
        m = np.asarray(in0, np.float32).reshape(p, -1)
        x = np.asarray(in1, np.float32).reshape(p, -1)
        d = (x + np.asarray(s0, np.float32)) ** 2
        P = np.cumsum(d.astype(np.float32), axis=1, dtype=np.float32)
        t = m - P + d
        U = np.minimum(np.minimum.accumulate(t, axis=1), np.float32(s1))
        return (U + P).astype(np.float32)

    def add_ramp_ref(in0, in1, s0, s1, imm2):
        p = in0.shape[0]
        a = np.asarray(in0, np.float32).reshape(p, -1)
        return (a + np.arange(a.shape[1], dtype=np.float32)
                * np.asarray(s0, np.float32)).astype(np.float32)

    d = sq(Src1 + C0)
    P = scan(AluOp.ADD, d)
    U = raw_scan(AluOp.MIN, (Src0 - P) + d, init=C1)
    specs = [
        ("DTW_ROW_ANT", Spec(body=U + P, reference=dtw_row_ref)),
        ("ADD_RAMP_ANT", Spec(body=Src0 + Idx * C0, reference=add_ramp_ref)),
    ]
    ops = []
    for name, spec in specs:
        if name in dom._SUB_OPCODE_FOR_NAME:
            ops.append(next(op for op in dom.OPS if op.name == name))
            continue
        row = dom._CUSTOM_DVE_ROW_BASE + len(dom.OPS)
        shas = {}
        for ver in ("v3", "v4"):
            s = DveOpSpec(name=name, opcode=row, uops=lower(spec, ver=ver),
                          rd1_en=_has_src1(spec))
            shas[ver] = s.sha(ver)
        op = DveOp(name, spec, subdim=False, uops_sha=shas)
        dom.OPS.append(op)
        dom._SUB_OPCODE_FOR_NAME[name] = row
        dom.CUSTOM_DVE_SPECS[name] = spec
        ops.append(op)
    return ops


DTW_ROW, ADD_RAMP = _register_ops()


# --- kernel builder -------------------------------------------------------- #

def _build_nc(reps: int = 1, gp_rows=(), ileave: bool = True) -> bass.Bass:
    """gp_rows: row indices (1..9) whose shifted-min runs on GPSIMD
    instead of DVE.  ileave pairs units so DVE work hides GPSIMD latency.
    reps > 1 replicates the schedule (slope-based timing)."""
    nc = bass.Bass("TRN2", debug=False, detect_race_conditions=False)
    x_d = nc.dram_tensor("x", [C, L], F32, kind="ExternalInput").ap()
    k_d = nc.dram_tensor("negk", [F, K], F32, kind="ExternalInput").ap()
    out_d = nc.dram_tensor("out", [C * F, NWC * NCHUNK], F32,
                           kind="ExternalOutput").ap()

    UNITS_R = UNITS * reps
    nu = len(UNITS_R)
    gp_rows = tuple(sorted(gp_rows))
    dve_rows = tuple(i for i in range(1, K) if i not in gp_rows)

    # --- op order / semaphore bookkeeping (python-side counts) ---
    # Per unit the DVE runs: [m~_i (dve rows)] + scan_i for i in 0..9,
    # then the extract.  With ileave, units are processed in pairs with
    # rows round-robined so GPSIMD m~ latency is hidden.
    # Sem counts: dve_sem +1 per scan & extract; gp_sem +1 per gp m~.
    dve_ops = []  # ("m"|"scan"|"ex", u, i)
    gp_ops = []   # ("m", u, i)

    def emit_unit_rows(units):
        for i in range(K):
            for u in units:
                if i > 0:
                    (gp_ops if i in gp_rows else dve_ops).append(("m", u, i))
                dve_ops.append(("scan", u, i))
        for u in units:
            dve_ops.append(("ex", u, 0))

    if ileave:
        for base in range(0, nu, 2):
            emit_unit_rows([base] + ([base + 1] if base + 1 < nu else []))
    else:
        for u in range(nu):
            emit_unit_rows([u])

    _scan_pos = {}
    _ex_pos = {}
    n = 0
    for kind, u, i in dve_ops:
        n += 1
        if kind == "scan":
            _scan_pos[(u, i)] = n
        elif kind == "ex":
            _ex_pos[u] = n
    _gp_pos = {}
    for n, (kind, u, i) in enumerate(gp_ops):
        _gp_pos[(u, i)] = n + 1

    def dve_through_scan(u, i):
        return _scan_pos[(u, i)]

    def dve_through_ex(u):
        return _ex_pos[u]

    def gp_through_m(u, i):
        return _gp_pos[(u, i)]

    with ExitStack() as ctx:
        sb = lambda shape, name: ctx.enter_context(
            nc.sbuf_tensor(name, shape, F32))
        X = [sb([128, L], f"Xt{cc}") for cc in range(2)]
        negK = sb([128, K], "negKt")
        m0t = sb([128, TFREE], "m0t")
        Y = [[sb([128, 1 + TFREE], f"Yt{s}_{i}") for i in range(2)]
             for s in range(SLOTS)]
        M = [sb([128, TFREE], f"Mt{s}") for s in range(SLOTS)]
        OB = [sb([128, NWC], f"OBt{s}") for s in range(SLOTS)]

        dma_sem = ctx.enter_context(nc.semaphore("dma_sem"))
        dma0_sem = ctx.enter_context(nc.semaphore("dma0_sem"))
        dve_sem = ctx.enter_context(nc.semaphore("dve_sem"))
        gp_sem = ctx.enter_context(nc.semaphore("gp_sem"))
        block = ctx.enter_context(nc.Block())

        def win_ap(cc, wc):
            xt = X[cc].ap()
            return bass.AP(xt.tensor, xt.offset + 5 * NWC * wc,
                           [list(xt.ap[0]), [5, NWC], [1, PROC]])

        def y_tail_ap(s):
            # column j=19 of each window of the final row (i=9 -> Y[s][1])
            yt = Y[s][1].ap()
            return bass.AP(yt.tensor, yt.offset + PROC,
                           [list(yt.ap[0]), [PROC, NWC]])

        @block.sync
        def _(sync):
            # negK + X0 first so cc0 compute starts before X1 lands.
            # X[cc] partition p holds x[4*cc + p//32, :] (source AP
            # replicates each channel row 32x via a step-0 dim)
            ksrc = bass.AP(k_d.tensor, 0, [[0, 4], [K, F], [1, K]])
            sync.dma_start(negK.ap(), ksrc).then_inc(dma0_sem, 16)
            for cc in range(2):
                src = bass.AP(x_d.tensor, 4 * cc * L,
                              [[L, 4], [0, 32], [1, L]])
                sync.dma_start(X[cc].ap(), src).then_inc(
                    dma0_sem if cc == 0 else dma_sem, 16)
            for u, (cc, wc) in enumerate(UNITS_R):
                s = u % SLOTS
                sync.wait_ge(dve_sem, dve_through_ex(u))
                sync.dma_start(
                    out_d[128 * cc:128 * (cc + 1),
                          NWC * wc:NWC * (wc + 1)],
                    OB[s].ap()).then_inc(dma_sem, 16)

        if gp_ops:
            @block.gpsimd
            def _(gpsimd):
                dve_waited = 0
                for kind, u, i in gp_ops:
                    s = u % SLOTS
                    # RAW: Y_{i-1}(u); WAR on M[s] vs scan_{i-1}... hmm
                    need = dve_through_scan(u, i - 1)
                    if need > dve_waited:
                        gpsimd.wait_ge(dve_sem, need)
                        dve_waited = need
                    yb = Y[s][(i - 1) % 2].ap()
                    gpsimd.tensor_tensor(
                        M[s].ap(), yb[:, 1:1 + TFREE], yb[:, 0:TFREE],
                        mybir.AluOpType.min).then_inc(gp_sem, 1)

        @block.vector
        def _(vector):
            # init: m0t = BIG except -w*GAMMA at each window start;
            # Y guard columns = BIG; OB[0] zeroed as the ramp source.
            vector.memset(m0t.ap(), BIG)
            vector.memset(OB[0].ap(), 0.0)
            for s in range(SLOTS):
                for i in range(2):
                    vector.memset(Y[s][i].ap()[:, 0:1], BIG)
            m0_seg = m0t.ap().rearrange("p (w s) -> p w s", s=PROC)
            vector._custom_dve(ADD_RAMP, out=m0_seg[:, :, 0],
                               in0=OB[0].ap(), s0=-GAMMA)
            vector.wait_ge(dma0_sem, 32)  # negK + X0
            gp_waited = 0
            dma_waited = 0
            x1_waited = False
            for kind, u, i in dve_ops:
                cc, wc = UNITS_R[u]
                s = u % SLOTS
                if cc == 1 and not x1_waited:
                    vector.wait_ge(dma_sem, 16)  # X1
                    x1_waited = True
                if kind == "m":
                    yb = Y[s][(i - 1) % 2].ap()
                    vector.tensor_tensor(
                        M[s].ap(), yb[:, 1:1 + TFREE], yb[:, 0:TFREE],
                        mybir.AluOpType.min).then_inc(dve_sem, 1)
                elif kind == "scan":
                    if i == 0:
                        m_ap = m0t.ap()
                    elif i in gp_rows:
                        need = gp_through_m(u, i)
                        if need > gp_waited:
                            vector.wait_ge(gp_sem, need)
                            gp_waited = need
                        m_ap = M[s].ap()
                    else:
                        m_ap = M[s].ap()
                    vector._custom_dve(
                        DTW_ROW, out=Y[s][i % 2].ap()[:, 1:1 + TFREE],
                        in0=m_ap, in1=win_ap(cc, wc),
                        s0=negK.ap()[:, i:i + 1],
                        s1=BIG).then_inc(dve_sem, 1)
                else:  # extract
                    if u >= SLOTS:
                        # WAR on OB[s]: out-DMA of unit u-SLOTS done
                        dneed = 16 * u
                        if dneed > dma_waited:
                            vector.wait_ge(dma_sem, dneed)
                            dma_waited = dneed
                    vector._custom_dve(
                        ADD_RAMP, out=OB[s].ap(), in0=y_tail_ap(s),
                        s0=GAMMA).then_inc(dve_sem, 1)

    lower_extended_insts(nc)
    return nc


_NC_CACHE = None


def _in_maps(x: np.ndarray, kernels: np.ndarray) -> list:
    x = np.ascontiguousarray(x, dtype=np.float32)
    negk = np.ascontiguousarray(-np.asarray(kernels, dtype=np.float32))
    return [{"x": x[b], "negk": negk} for b in range(B)]


def kernel(x: np.ndarray, kernels: np.ndarray) -> np.ndarray:
    global _NC_CACHE
    if _NC_CACHE is None:
        _NC_CACHE = _build_nc()
    nc = _NC_CACHE
    in_maps = _in_maps(x, kernels)
    res = run_bass_kernel_spmd(nc, in_maps, core_ids=list(range(B)))
    out = np.stack([res.results[b]["out"] for b in range(B)], axis=0)
    return out[:, :, :NW]


# revision 23
# speedup vs baseline: 2.0883x; 2.0883x over previous
"""DTW layer (short kernel) Trainium2 Bass kernel — custom-DVE-op version.

Problem: x (B=8, C=8, L=4096) f32, kernels (F=32, K=10) f32.
For each (b, c, f, w): DTW cost between kernels[f] (len 10) and window
x[b, c, 5w : 5w+20], w in [0, 815).  Output (B, C*F, 815) f32.

Sharding: data-parallel over batch — core b computes batch b entirely
(C*F = 256 (c,f) combos = 2 partition chunks of 128).

Algorithm: within one window the DTW row update
    acc[j] = d[j] + min(min(up, diag), acc[j-1]),  acc[-1] = +inf
unrolls (min-plus over the "left" direction) to
    acc[j] = min_{i<=j}( m[i] - P[i-1] ) + P[j],   P = prefix sum of d,
with m[i] = min(prev[i], prev[i-1]) the up/diag term.  All windows of a
chunk are streamed back-to-back (20 cells per window, NO separators).
Window isolation comes from a staircase: the carried row values
Y = acc - w*GAMMA drop by GAMMA per window, so one global min-scan
never lets an earlier window undercut a later one (the global prefix
sum P is monotone and only strengthens the isolation); GAMMA must
exceed the worst DTW-value spread within a chunk (4096 here; 2048 was
observed to fail).

ScalarE pre-squares d = (x - k_i)^2 into ping-pong tiles one row ahead.
One custom DVE instruction per row (DTW_ROWD_ANT) then does everything
at ~1 elem/cycle — vs 2 cycles/elem for the stock tensor_tensor_scan —
including the shifted min, so there is no separate m~ pass at all:
    P   = scan(ADD, d)                  d = Src1 (pre-squared x)
    h   = Y_prev - P                    Y_prev = Src0
    D   = delay1(h)                     = h[k-1]  (swap-flop trick)
    U   = scan(MIN, min(h + d, D), C1)  C1 = 1e30
    out = U + P                         = acc_i[k] - w*GAMMA
Two hand-applied uop techniques make this a single instruction:
 - nested scans: the MIN-scan's expr references the ADD-scan.
   dve_spec's scan() forbids that syntactically, but the datapath
   handles it (each scan stage has its own CURR_ALU_OUT feedback
   flop), so the Scan node is built directly, bypassing __post_init__.
 - a one-element delay register: a BYPASS stage with swap_enable
   outputs the swap flop while latching operand B (HW spec 3.5: for
   BYPASS(a), swap captures b), giving h[k-1]; the seed uop preloads
   the flop with C1.  lower() emits a MAX(C1, h) placeholder stage
   that patch_delay() rewrites.
min(h[k]+d[k], h[k-1]) == min(Y[k], Y[k-1]) - P[k-1] exactly, window
boundaries included (the staircase dominates the cross-window term).
Row 0 uses a no-delay variant reading a constant m~0 tile
(-w*GAMMA at each window start, BIG elsewhere).  A tiny ADD_RAMP op
(out = in + Idx*C0) undoes the staircase when extracting column j=19
of each window; CONST_FILL/RAMP_SEL build m~0.

Hard-won toolchain facts encoded below:
 - lower_extended_insts() must run after tracing (raw Bass skips
   Bacc's codegen_inst_isa_subclasses pass; without it custom-DVE
   instructions have empty .instr bytes -> walrus "ISA wrong length").
 - engine Memset instructions do NOT execute in engine program order
   (a following DVE read sees stale SBUF), so all constant tiles are
   built with select-based custom ops whose pacing input can be
   garbage/NaN without poisoning the result.
 - semaphores persist across NEFF executions: they are cleared in
   `main` behind an all-engine barrier (its gather/release pair is
   self-resetting), and the DVE holds the program end until every DMA
   and ACT completion so the runtime cannot return early.
 - walrus rejects tensor ops and InstPool on the Pool engine, and at
   most 2 embedded sync-waits per instruction: engines are programmed
   raw with standalone wait_ge and per-engine semaphores.
"""

from contextlib import ExitStack

import numpy as np

import concourse.bass as bass
import concourse.mybir as mybir
from concourse.bass_utils import run_bass_kernel_spmd
from concourse.library_overlay import lower_extended_insts

# Problem constants (hardcoded per harness contract)
B, C, L = 8, 8, 4096
F, K = 32, 10
PROC, STEP = 20, 5
NW = 815          # windows actually computed == chan_outlen
NWTOT = 816       # allocated window columns (= NWC * NCHUNK)
BIG = 1e30

F32 = mybir.dt.float32


# --- custom DVE ops -------------------------------------------------------- #

def _register_ops():
    from dataclasses import dataclass

    import concourse.dve_ops as dom
    from concourse.dve_ops import DveOp
    from concourse.dve_spec import (
        C0, C1, C2, AluOp, Idx, Scan, Spec, Src0, Src1, _has_src1, lower,
        scan, sq,
    )
    from concourse.dve_spec import SubIdx as SubIdx_, Zero as Zero_
    from concourse.dve_uop import AluInp, DveOpSpec

    def raw_scan(op, expr, init=None):
        s = object.__new__(Scan)
        object.__setattr__(s, "op", op)
        object.__setattr__(s, "expr", expr)
        object.__setattr__(s, "init", init)
        object.__setattr__(s, "_subdim_step", None)
        return s

    def dtw_row_ref(in0, in1, s0, s1, imm2):
        p = in0.shape[0]
        m = np.asarray(in0, np.float32).reshape(p, -1)
        x = np.asarray(in1, np.float32).reshape(p, -1)
        d = (x + np.asarray(s0, np.float32)) ** 2
        P = np.cumsum(d.astype(np.float32), axis=1, dtype=np.float32)
        t = m - P + d
        U = np.minimum(np.minimum.accumulate(t, axis=1), np.float32(s1))
        return (U + P).astype(np.float32)

    def dtw_row0_ref(in0, in1, s0, s1, imm2):
        p = in0.shape[0]
        m = np.asarray(in0, np.float32).reshape(p, -1)
        d = np.asarray(in1, np.float32).reshape(p, -1)
        P = np.cumsum(d, axis=1, dtype=np.float32)
        t = m - P + d
        U = np.minimum(np.minimum.accumulate(t, axis=1), np.float32(s1))
        return (U + P).astype(np.float32)

    def dtw_rowd_ref(in0, in1, s0, s1, imm2):
        # in0 = Y_prev, in1 = d = (x - k_i)^2 (pre-squared by ScalarE).
        # h = Y - P;  e2[k] = min(h[k] + d[k], h[k-1]);  out = minscan(e2)+P
        p = in0.shape[0]
        y = np.asarray(in0, np.float32).reshape(p, -1)
        d = np.asarray(in1, np.float32).reshape(p, -1)
        P = np.cumsum(d, axis=1, dtype=np.float32)
        h = y - P
        hprev = np.concatenate(
            [np.full((p, 1), np.float32(s1)), h[:, :-1]], axis=1)
        e2 = np.minimum(h + d, hprev)
        U = np.minimum(np.minimum.accumulate(e2, axis=1), np.float32(s1))
        return (U + P).astype(np.float32)

    def add_ramp_ref(in0, in1, s0, s1, imm2):
        p = in0.shape[0]
        a = np.asarray(in0, np.float32).reshape(p, -1)
        return (a + np.arange(a.shape[1], dtype=np.float32)
                * np.asarray(s0, np.float32)).astype(np.float32)

    def patch_delay(uops):
        """Rewrite the MAX(C1, h) proxy stage into a one-element delay:
        steady: BYPASS outputs the swap flop (h[k-1]) while swap_enable
        captures operand B (h[k]); seed: capture C1 (big) into the flop.
        HW §3.5: for BYPASS(a), swap latches b."""
        assert len(uops) == 2, f"expected seed+steady, got {len(uops)}"
        seed, steady = uops
        s_d = [i for i, dp in enumerate(steady.datapath_config)
               if dp.op == AluOp.MAX]
        assert len(s_d) == 1, f"proxy MAX stage not unique: {s_d}"
        s_d = s_d[0]
        st = steady.datapath_config[s_d]
        c1_loc, h_loc = st.alu_src0, st.alu_src1
        st.op = AluOp.BYPASS
        st.alu_src0 = AluInp.CURR_SWAP_OUT
        st.alu_src1 = h_loc
        st.swap_enable = 1
        sd = seed.datapath_config[s_d]
        sd.op = AluOp.BYPASS
        sd.alu_src0 = c1_loc
        sd.alu_src1 = c1_loc
        sd.swap_enable = 1

    @dataclass(frozen=True)
    class DelayDveOp(DveOp):
        def compile(self, ver):
            key = (self.name, ver)
            if (r := dom._COMPILE_CACHE.get(key)) is not None:
                return r
            uops = lower(self.spec, ver=ver)
            patch_delay(uops)
            result = DveOpSpec(
                name=self.name, opcode=dom.get_dve_sub_opcode(self.name),
                uops=uops, rd1_en=_has_src1(self.spec))
            got = result.sha(ver)
            if self.uops_sha.get(ver) != got:
                raise ValueError(f"{self.name}: sha drift {got}")
            dom._COMPILE_CACHE[key] = result
            return result

    # original 2-port row op (raw x + per-partition bias): kept for row 0
    # fallback and tests
    d = sq(Src1 + C0)
    P = scan(AluOp.ADD, d)
    U = raw_scan(AluOp.MIN, (Src0 - P) + d, init=C1)
    spec_row = Spec(body=U + P, reference=dtw_row_ref)

    # row 0 with pre-squared costs: in0 = m~0 const, in1 = d
    P0 = scan(AluOp.ADD, Src1)
    U0 = raw_scan(AluOp.MIN, (Src0 - P0) + Src1, init=C1)
    spec_row0 = Spec(body=U0 + P0, reference=dtw_row0_ref)

    # rows 1..9, shifted-min fused via the swap-flop delay: in0 = Y_prev,
    # in1 = d.  The MAX node is a placeholder patched by patch_delay.
    from concourse.dve_spec import Bin

    Pd = scan(AluOp.ADD, Src1)
    h = Src0 - Pd
    Dl = Bin(AluOp.MAX, C1, h)
    e2 = Bin(AluOp.MIN, h + Src1, Dl)
    Ud = raw_scan(AluOp.MIN, e2, init=C1)
    spec_rowd = Spec(body=Ud + Pd, reference=dtw_rowd_ref)

    spec_ramp = Spec(body=Src0 + Idx * C0, reference=add_ramp_ref)

    # Position-pattern writers for m~0.  in0 is only a pacing stream (a
    # view of the destination tile); its values are never selected, so
    # garbage/NaN in the tile cannot poison the result — and unlike the
    # engine Memset instruction these execute in DVE program order.
    from concourse.dve_spec import One, select

    def const_fill_ref(in0, in1, s0, s1, imm2):
        a = np.asarray(in0, np.float32)
        return np.full_like(a, np.float32(s1))

    def ramp_sel_ref(in0, in1, s0, s1, imm2):
        p = in0.shape[0]
        a = np.asarray(in0, np.float32).reshape(p, -1)
        return np.broadcast_to(
            np.arange(a.shape[1], dtype=np.float32) * np.float32(s0),
            a.shape).astype(np.float32)

    spec_fill = Spec(body=select(One, C1, Src0), reference=const_fill_ref)
    spec_rampsel = Spec(body=select(One, Idx * C0, Src0),
                        reference=ramp_sel_ref)

    specs = [
        ("DTW_ROW_ANT", spec_row, DveOp, False),
        ("ADD_RAMP_ANT", spec_ramp, DveOp, False),
        ("DTW_ROW0_ANT", spec_row0, DveOp, False),
        ("DTW_ROWD_ANT", spec_rowd, DelayDveOp, False),
        ("CONST_FILL_ANT", spec_fill, DveOp, False),
        ("RAMP_SEL_ANT", spec_rampsel, DveOp, False),
    ]
    ops = []
    for name, spec, cls, subdim in specs:
        if name in dom._SUB_OPCODE_FOR_NAME:
            ops.append(next(op for op in dom.OPS if op.name == name))
            continue
        row = dom._CUSTOM_DVE_ROW_BASE + len(dom.OPS)
        shas = {}
        for ver in ("v3", "v4"):
            uops = lower(spec, ver=ver)
            if cls is DelayDveOp:
                patch_delay(uops)
            s = DveOpSpec(name=name, opcode=row, uops=uops,
                          rd1_en=_has_src1(spec))
            shas[ver] = s.sha(ver)
        op = cls(name, spec, subdim=subdim, uops_sha=shas)
        dom.OPS.append(op)
        dom._SUB_OPCODE_FOR_NAME[name] = row
        dom.CUSTOM_DVE_SPECS[name] = spec
        ops.append(op)
    return ops


DTW_ROW, ADD_RAMP, DTW_ROW0, DTW_ROWD, CONST_FILL, RAMP_SEL = _register_ops()


# --- kernel builder -------------------------------------------------------- #

def _build_nc(reps: int = 1, nwc: int = 136,
              gamma: float = 4096.0, gs: int = 2) -> bass.Bass:
    """Units are processed in pairs (same cc); ScalarE pre-squares
    (x - k_i)^2 for the pair's x-span into ping-pong Xs tiles one row
    ahead of the DVE scans.  nwc = windows per chunk (staircase resets
    per chunk, so nwc*gamma bounds the magnitude fp32 must carry);
    gamma must exceed the max DTW-value spread within a chunk stream.
    reps > 1 replicates the schedule (slope-based timing)."""
    NWC = nwc
    GAMMA = gamma
    NCHUNK = NWTOT // NWC
    assert NWC * NCHUNK == NWTOT
    TFREE = NWC * PROC
    UNITS = [(cc, wc) for cc in range(2) for wc in range(NCHUNK)]
    nc = bass.Bass("TRN2", debug=False, detect_race_conditions=False)
    x_d = nc.dram_tensor("x", [C, L], F32, kind="ExternalInput").ap()
    k_d = nc.dram_tensor("negk", [F, K], F32, kind="ExternalInput").ap()
    out_d = nc.dram_tensor("out", [C * F, NWTOT], F32,
                           kind="ExternalOutput").ap()

    UNITS_R = UNITS * reps
    nu = len(UNITS_R)
    GS = gs  # units per group (same cc; ACT tiles are per-group-row)
    SLOTS = GS
    assert (NCHUNK % GS) == 0  # groups never straddle a cc boundary
    SPAN = 5 * NWC * GS + PROC - 5  # x columns one group's windows touch
    groups = [list(range(b, min(b + GS, nu))) for b in range(0, nu, GS)]

    # --- op order / semaphore bookkeeping (python-side counts) ---
    dve_ops = []  # ("scan"|"ex", u, i)
    act_ops = []  # (g, i)
    for g, units in enumerate(groups):
        for i in range(K):
            act_ops.append((g, i))
            for u in units:
                dve_ops.append(("scan", u, i))
        for u in units:
            dve_ops.append(("ex", u, 0))

    _scan_pos, _ex_pos = {}, {}
    n = 0
    for kind, u, i in dve_ops:
        n += 1
        if kind == "scan":
            _scan_pos[(u, i)] = n
        else:
            _ex_pos[u] = n
    _act_pos = {gi: n + 1 for n, gi in enumerate(act_ops)}

    with ExitStack() as ctx:
        sb = lambda shape, name: ctx.enter_context(
            nc.sbuf_tensor(name, shape, F32))
        X = [sb([128, L], f"Xt{cc}") for cc in range(2)]
        Xs = [sb([128, SPAN], f"Xsq{j}") for j in range(2)]
        negK = sb([128, K], "negKt")
        m0t = sb([128, TFREE], "m0t")
        Y = [[sb([128, TFREE], f"Yt{s}_{i}") for i in range(2)]
             for s in range(SLOTS)]
        OB = [sb([128, NWC], f"OBt{s}") for s in range(SLOTS)]

        dma_sem = ctx.enter_context(nc.semaphore("dma_sem"))
        dma0_sem = ctx.enter_context(nc.semaphore("dma0_sem"))
        dve_sem = ctx.enter_context(nc.semaphore("dve_sem"))
        act_sem = ctx.enter_context(nc.semaphore("act_sem"))

        # Semaphores persist across NEFF executions (nothing resets them
        # between dispatches of a loaded executable).  Clear them in the
        # `main` block before any engine branches into its program, with
        # an all-engine barrier (whose gather/release pair is
        # self-resetting) fencing every engine's first semaphore use.
        for sem in (dma0_sem, dma_sem, act_sem, dve_sem):
            nc.sync.sem_clear(sem)
        nc.all_engine_barrier()

        block = ctx.enter_context(nc.Block())

        def win_sq(i, wc):
            xt = Xs[i % 2].ap()
            return bass.AP(xt.tensor, xt.offset + 5 * NWC * (wc % GS),
                           [list(xt.ap[0]), [5, NWC], [1, PROC]])

        def y_tail_ap(s):
            # column j=19 of each window of the final row (i=9 -> Y[s][1])
            yt = Y[s][1].ap()
            return bass.AP(yt.tensor, yt.offset + PROC - 1,
                           [list(yt.ap[0]), [PROC, NWC]])

        @block.sync
        def _(sync):
            # negK + X0 first so cc0 compute starts before X1 lands.
            # X[cc] partition p holds x[4*cc + p//32, :] (source AP
            # replicates each channel row 32x via a step-0 dim)
            ksrc = bass.AP(k_d.tensor, 0, [[0, 4], [K, F], [1, K]])
            sync.dma_start(negK.ap(), ksrc).then_inc(dma0_sem, 16)
            for cc in range(2):
                src = bass.AP(x_d.tensor, 4 * cc * L,
                              [[L, 4], [0, 32], [1, L]])
                sync.dma_start(X[cc].ap(), src).then_inc(
                    dma0_sem if cc == 0 else dma_sem, 16)
            for u, (cc, wc) in enumerate(UNITS_R):
                s = u % SLOTS
                sync.wait_ge(dve_sem, _ex_pos[u])
                sync.dma_start(
                    out_d[128 * cc:128 * (cc + 1),
                          NWC * wc:NWC * (wc + 1)],
                    OB[s].ap()).then_inc(dma_sem, 16)

        @block.scalar
        def _(scalar):
            scalar.wait_ge(dma0_sem, 32)  # negK + X0
            dve_waited = 0
            x1_waited = False
            for g, i in act_ops:
                cc, _ = UNITS_R[groups[g][0]]
                if cc == 1 and not x1_waited:
                    scalar.wait_ge(dma_sem, 16)  # X1
                    x1_waited = True
                # WAR on Xs[i%2]: previous readers are the row (i-2)
                # scans of this group (or rows 8/9 of the previous group)
                if i >= 2:
                    need = _scan_pos[(groups[g][-1], i - 2)]
                elif g > 0:
                    need = _scan_pos[(groups[g - 1][-1], 8 + i)]
                else:
                    need = 0
                if need > dve_waited:
                    scalar.wait_ge(dve_sem, need)
                    dve_waited = need
                wc0 = UNITS_R[groups[g][0]][1]
                base = 5 * NWC * wc0
                scalar.activation(
                    Xs[i % 2].ap(), X[cc].ap()[:, base:base + SPAN],
                    mybir.ActivationFunctionType.Square,
                    bias=negK.ap()[:, i:i + 1],
                    scale=1.0).then_inc(act_sem, 1)

        @block.vector
        def _(vector):
            # init: m0t = -w*GAMMA at each window start, BIG elsewhere.
            # Two position-based custom ops — engine Memset instructions
            # do NOT execute in DVE program order on this toolchain
            # (observed: a following read sees stale SBUF), so no memsets.
            m0_seg = m0t.ap().rearrange("p (w s) -> p w s", s=PROC)
            vector._custom_dve(CONST_FILL, out=m0_seg[:, :, 1:],
                               in0=m0_seg[:, :, 1:], s1=BIG)
            vector._custom_dve(RAMP_SEL, out=m0_seg[:, :, 0],
                               in0=m0_seg[:, :, 0], s0=-GAMMA)
            act_waited = 0
            dma_waited = 0
            for kind, u, i in dve_ops:
                cc, wc = UNITS_R[u]
                s = u % SLOTS
                g = u // GS
                if kind == "scan":
                    need = _act_pos[(g, i)]
                    if need > act_waited:
                        vector.wait_ge(act_sem, need)
                        act_waited = need
                    if i == 0:
                        vector._custom_dve(
                            DTW_ROW0, out=Y[s][0].ap(), in0=m0t.ap(),
                            in1=win_sq(i, wc),
                            s1=BIG).then_inc(dve_sem, 1)
                    else:
                        vector._custom_dve(
                            DTW_ROWD, out=Y[s][i % 2].ap(),
                            in0=Y[s][(i - 1) % 2].ap(),
                            in1=win_sq(i, wc),
                            s1=BIG).then_inc(dve_sem, 1)
                else:  # extract
                    if u >= SLOTS:
                        # WAR on OB[s]: out-DMA of unit u-SLOTS done
                        dneed = 16 * (u - SLOTS + 2)
                        if dneed > dma_waited:
                            vector.wait_ge(dma_sem, dneed)
                            dma_waited = dneed
                    vector._custom_dve(
                        ADD_RAMP, out=OB[s].ap(), in0=y_tail_ap(s),
                        s0=GAMMA).then_inc(dve_sem, 1)
            # Semaphores persist across NEFF executions; without a final
            # clear a second dispatch of the same executable sees every
            # wait_ge already satisfied and the engines race.  The DVE is
            # the last producer: wait for the other engines' final counts
            # (their last waits all precede their final increments), then
            # zero everything for the next dispatch.
            # Completion fence: hold the DVE (the last engine to halt)
            # until every DMA and ACT op has finished, so the runtime
            # cannot hand the output buffer back while the final
            # out-DMAs are still in flight.
            vector.wait_ge(dma0_sem, 32)
            vector.wait_ge(dma_sem, 16 * (1 + nu))
            vector.wait_ge(act_sem, len(act_ops))

    lower_extended_insts(nc)
    return nc


_NC_CACHE = None


def _in_maps(x: np.ndarray, kernels: np.ndarray) -> list:
    x = np.ascontiguousarray(x, dtype=np.float32)
    negk = np.ascontiguousarray(-np.asarray(kernels, dtype=np.float32))
    return [{"x": x[b], "negk": negk} for b in range(B)]


def kernel(x: np.ndarray, kernels: np.ndarray) -> np.ndarray:
    global _NC_CACHE
    if _NC_CACHE is None:
        _NC_CACHE = _build_nc()
    nc = _NC_CACHE
    in_maps = _in_maps(x, kernels)
    res = run_bass_kernel_spmd(nc, in_maps, core_ids=list(range(B)))
    out = np.stack([res.results[b]["out"] for b in range(B)], axis=0)
    return out[:, :, :NW]
